# revision 2
# baseline (speedup 1.0000x reference)
"""Trainium2 Bass kernel for the CoSSL retrieval/hard-negative-mining module.

Reference computation (B=256, D=128, R=2304, Q=65536, TOPK=5):
    qn = l2norm(q); kn = l2norm(k)
    score_batch = qn @ kn.T                      [B, B]
    score_queue = qn @ moco_queue                [B, Q]
    score_ref   = ref_feats @ ref_queue          [B, Q]
    mask_eq     = indices[:,None] == index_queue [B, Q]
    top5        = topk(where(mask_eq, -inf, score_ref), 5)
    score_queue = score_queue * score_ref * (+1 at top5 else -1)
    mask_queue  = mask_eq.astype(i32) with top5 set to 1
    return concat([score_batch, score_queue], 1), concat([mask_batch, mask_queue], 1)

Sharding: queues column-sharded across 8 NeuronCores (8192 cols each).
Each core computes its slice of score_queue/score_ref/mask and the
device-local top-8-per-512-chunk candidates (value + index) of the masked
score_ref via DVE max/max_index. The host merges the per-core candidates,
rescores the few survivors exactly in float64 (distributed top-k merge),
and patches the +-1 sign / mask at the 5 winning positions per row.

Matmul precision: the big score_ref matmul runs in float32r (full PE rate,
~1.6e-4 relative error - plenty for the output values; candidate *ranking*
is made exact by the float64 host rescore of ~32 candidates/row).
score_queue / score_batch run in native fp32.
"""

import sys

for _p in ("/opt/trn_rl_repo",):
    if _p not in sys.path:
        sys.path.insert(0, _p)

import numpy as np

import concourse.bass as bass
import concourse.mybir as mybir
import concourse.tile as tile
from concourse import bacc
from concourse.bass_utils import run_bass_kernel_spmd
from concourse.masks import make_identity

B = 256
D = 128
R = 2304
Q = 65536
NCORES = 8
QS = Q // NCORES          # 8192 columns per core
CH = 512                  # free-dim chunk (one PSUM bank)
NCH = QS // CH            # 16 chunks
KT = R // 128             # 18 contraction tiles
TOPK = 5
NEG_BIG = -1.0e30

F32 = mybir.dt.float32
F32R = mybir.dt.float32r
I32 = mybir.dt.int32
U32 = mybir.dt.uint32

# set True (e.g. from test.py) to capture an NTFF profile; exec time lands in
# LAST_EXEC_NS after each kernel() call.
TRACE = False
LAST_EXEC_NS = None

_CACHED = {}


def _build():
    nc = bacc.Bacc("TRN2", target_bir_lowering=False, debug=False)

    refq_d = nc.dram_tensor("refq", [R, QS], F32, kind="ExternalInput")
    moco_d = nc.dram_tensor("moco", [D, QS], F32, kind="ExternalInput")
    iq_d = nc.dram_tensor("iq", [1, QS], F32, kind="ExternalInput")
    idx_d = nc.dram_tensor("idx", [B, 1], F32, kind="ExternalInput")
    idxrow_d = nc.dram_tensor("idxrow", [1, B], F32, kind="ExternalInput")
    q_d = nc.dram_tensor("q", [B, D], F32, kind="ExternalInput")
    k_d = nc.dram_tensor("k", [B, D], F32, kind="ExternalInput")
    refT_d = nc.dram_tensor("refT", [R, B], F32, kind="ExternalInput")

    prod_d = nc.dram_tensor("prod", [B, QS], F32, kind="ExternalOutput")
    maskq_d = nc.dram_tensor("maskq", [B, QS], I32, kind="ExternalOutput")
    cvals_d = nc.dram_tensor("cvals", [B, NCH * 8], F32, kind="ExternalOutput")
    cidx_d = nc.dram_tensor("cidx", [B, NCH * 8], U32, kind="ExternalOutput")
    sb_d = nc.dram_tensor("sb", [B, B], F32, kind="ExternalOutput")
    maskb_d = nc.dram_tensor("maskb", [B, B], I32, kind="ExternalOutput")

    with tile.TileContext(nc) as tc:
        with tc.tile_pool(name="const", bufs=1) as cpool, \
             tc.tile_pool(name="refrhs", bufs=2) as refpool, \
             tc.tile_pool(name="mocorhs", bufs=2) as mocopool, \
             tc.tile_pool(name="work", bufs=3) as wpool, \
             tc.tile_pool(name="outstage", bufs=3) as opool, \
             tc.tile_pool(name="psum_sr", bufs=2, space="PSUM") as srpsum, \
             tc.tile_pool(name="psum_sq", bufs=2, space="PSUM") as sqpsum, \
             tc.tile_pool(name="psum_misc", bufs=2, space="PSUM") as mpsum:

            # ---- small persistent tensors -------------------------------
            iq_s = cpool.tile([128, QS], F32, tag="iq")
            nc.sync.dma_start(out=iq_s[:], in_=iq_d[:].partition_broadcast(128))

            idx_s = []          # per m-tile [128,1] per-partition scalars
            for m in range(2):
                t = cpool.tile([128, 1], F32, tag=f"idx{m}")
                nc.sync.dma_start(out=t[:], in_=idx_d[m * 128:(m + 1) * 128, :])
                idx_s.append(t)

            idxrow_s = cpool.tile([128, B], F32, tag="idxrow")
            nc.sync.dma_start(out=idxrow_s[:],
                              in_=idxrow_d[:].partition_broadcast(128))

            lhsT = cpool.tile([128, KT * B], F32R, tag="lhsT")
            for kt in range(KT):
                nc.sync.dma_start(
                    out=lhsT[:, kt * B:(kt + 1) * B],
                    in_=refT_d[kt * 128:(kt + 1) * 128, :].bitcast(F32R))

            ident = cpool.tile([128, 128], F32, tag="ident")
            make_identity(nc, ident[:])

            # ---- normalize q,k and build transposed copies --------------
            qnT = cpool.tile([128, B], F32, tag="qnT")
            knT = cpool.tile([128, B], F32, tag="knT")
            for (src_d, dstT) in ((q_d, qnT), (k_d, knT)):
                for m in range(2):
                    raw = wpool.tile([128, D], F32, tag="rawqk")
                    nc.sync.dma_start(out=raw[:],
                                      in_=src_d[m * 128:(m + 1) * 128, :])
                    sqv = wpool.tile([128, D], F32, tag="sqv")
                    ssum = wpool.tile([128, 1], F32, tag="ssum")
                    nc.scalar.activation(
                        out=sqv[:], in_=raw[:],
                        func=mybir.ActivationFunctionType.Square,
                        accum_out=ssum[:])
                    rec = wpool.tile([128, 1], F32, tag="rec")
                    nc.vector.reciprocal(out=rec[:], in_=ssum[:])
                    inv = wpool.tile([128, 1], F32, tag="inv")
                    nc.scalar.sqrt(out=inv[:], in_=rec[:])
                    nrm = wpool.tile([128, D], F32, tag="nrm")
                    nc.vector.tensor_scalar_mul(nrm[:], raw[:], inv[:])
                    pt = mpsum.tile([128, 128], F32)
                    nc.tensor.transpose(pt[:], nrm[:], ident[:])
                    nc.scalar.copy(out=dstT[:, m * 128:(m + 1) * 128], in_=pt[:])

            # ---- score_batch + mask_batch -------------------------------
            for m in range(2):
                psb = mpsum.tile([128, B], F32)
                nc.tensor.matmul(psb[:], qnT[:, m * 128:(m + 1) * 128], knT[:],
                                 start=True, stop=True)
                sb_s = opool.tile([128, B], F32, tag="sb")
                nc.scalar.copy(out=sb_s[:], in_=psb[:])
                nc.sync.dma_start(out=sb_d[m * 128:(m + 1) * 128, :], in_=sb_s[:])

                mb_s = opool.tile([128, B], I32, tag="mb")
                nc.vector.tensor_scalar(mb_s[:], idxrow_s[:], idx_s[m][:], None,
                                        op0=mybir.AluOpType.is_equal)
                nc.sync.dma_start(out=maskb_d[m * 128:(m + 1) * 128, :],
                                  in_=mb_s[:])

            # ---- candidate accumulators ---------------------------------
            cv_s = [cpool.tile([128, NCH * 8], F32, tag=f"cv{m}",
                               name=f"cv{m}") for m in range(2)]
            ci_s = [cpool.tile([128, NCH * 8], U32, tag=f"ci{m}",
                               name=f"ci{m}") for m in range(2)]

            # ---- main streaming loop ------------------------------------
            for n in range(NCH):
                csl = slice(n * CH, (n + 1) * CH)

                rhs_ref = refpool.tile([128, KT * CH], F32R, tag="rhsref")
                for kt in range(KT):
                    nc.sync.dma_start(
                        out=rhs_ref[:, kt * CH:(kt + 1) * CH],
                        in_=refq_d[kt * 128:(kt + 1) * 128, csl].bitcast(F32R))
                rhs_moco = mocopool.tile([128, CH], F32, tag="rhsmoco")
                nc.sync.dma_start(out=rhs_moco[:], in_=moco_d[:, csl])

                for m in range(2):
                    msl = slice(m * 128, (m + 1) * 128)

                    psq = sqpsum.tile([128, CH], F32)
                    nc.tensor.matmul(psq[:], qnT[:, msl], rhs_moco[:],
                                     start=True, stop=True)

                    psr = srpsum.tile([128, CH], F32)
                    for kt in range(KT):
                        nc.tensor.matmul(
                            psr[:],
                            lhsT[:, kt * B + m * 128: kt * B + (m + 1) * 128],
                            rhs_ref[:, kt * CH:(kt + 1) * CH],
                            start=(kt == 0), stop=(kt == KT - 1))

                    sq_neg = wpool.tile([128, CH], F32, tag="sqneg")
                    nc.scalar.copy(out=sq_neg[:], in_=psq[:])
                    nc.scalar.mul(out=sq_neg[:], in_=sq_neg[:], mul=-1.0)

                    mq_s = opool.tile([128, CH], I32, tag="mq")
                    nc.vector.tensor_scalar(mq_s[:], iq_s[:, csl], idx_s[m][:],
                                            None, op0=mybir.AluOpType.is_equal)
                    nc.sync.dma_start(out=maskq_d[msl, csl], in_=mq_s[:])

                    m_scaled = wpool.tile([128, CH], F32, tag="mscaled")
                    nc.vector.tensor_scalar(m_scaled[:], iq_s[:, csl],
                                            idx_s[m][:], NEG_BIG,
                                            op0=mybir.AluOpType.is_equal,
                                            op1=mybir.AluOpType.mult)

                    masked = wpool.tile([128, CH], F32, tag="masked")
                    nc.vector.tensor_tensor(masked[:], psr[:], m_scaled[:],
                                            op=mybir.AluOpType.add)

                    prod_s = opool.tile([128, CH], F32, tag="prod")
                    nc.vector.tensor_tensor(prod_s[:], psr[:], sq_neg[:],
                                            op=mybir.AluOpType.mult)
                    nc.sync.dma_start(out=prod_d[msl, csl], in_=prod_s[:])

                    nc.vector.max(out=cv_s[m][:, n * 8:(n + 1) * 8],
                                  in_=masked[:])
                    nc.vector.max_index(out=ci_s[m][:, n * 8:(n + 1) * 8],
                                        in_max=cv_s[m][:, n * 8:(n + 1) * 8],
                                        in_values=masked[:])

            for m in range(2):
                msl = slice(m * 128, (m + 1) * 128)
                nc.sync.dma_start(out=cvals_d[msl, :], in_=cv_s[m][:])
                nc.sync.dma_start(out=cidx_d[msl, :], in_=ci_s[m][:])

    nc.finalize()
    return nc


def _get_nc():
    if "nc" not in _CACHED:
        _CACHED["nc"] = _build()
    return _CACHED["nc"]


def kernel(q, k, ref_feats, moco_queue, ref_queue, indices, index_queue):
    global LAST_EXEC_NS
    q = np.ascontiguousarray(q, dtype=np.float32)
    k = np.ascontiguousarray(k, dtype=np.float32)
    ref_feats = np.ascontiguousarray(ref_feats, dtype=np.float32)
    moco_queue = np.ascontiguousarray(moco_queue, dtype=np.float32)
    ref_queue = np.ascontiguousarray(ref_queue, dtype=np.float32)
    idx_i = np.asarray(indices)
    iq_i = np.asarray(index_queue)

    nc = _get_nc()

    idx_f = idx_i.astype(np.float32).reshape(B, 1)
    idxrow_f = idx_i.astype(np.float32).reshape(1, B)
    refT = np.ascontiguousarray(ref_feats.T)

    in_maps = []
    for c in range(NCORES):
        sl = slice(c * QS, (c + 1) * QS)
        in_maps.append({
            "refq": np.ascontiguousarray(ref_queue[:, sl]),
            "moco": np.ascontiguousarray(moco_queue[:, sl]),
            "iq": iq_i[sl].astype(np.float32).reshape(1, QS),
            "idx": idx_f,
            "idxrow": idxrow_f,
            "q": q,
            "k": k,
            "refT": refT,
        })

    kwargs = {}
    if TRACE:
        kwargs.update(trace=True, trace_cores=list(range(NCORES)))
    res = run_bass_kernel_spmd(nc, in_maps, core_ids=list(range(NCORES)),
                               **kwargs)
    LAST_EXEC_NS = res.exec_time_ns
    outs = res.results

    score = np.empty((B, B + Q), dtype=np.float32)
    mask = np.empty((B, B + Q), dtype=np.int32)
    score[:, :B] = outs[0]["sb"]
    mask[:, :B] = outs[0]["maskb"]
    for c in range(NCORES):
        sl = slice(B + c * QS, B + (c + 1) * QS)
        score[:, sl] = outs[c]["prod"]
        mask[:, sl] = outs[c]["maskq"]

    # ---- distributed top-k merge --------------------------------------
    # candidates: per core, per 512-chunk, top-8 (value, in-chunk index)
    vals = np.concatenate([outs[c]["cvals"] for c in range(NCORES)], axis=1)
    gidx = np.concatenate(
        [(c * QS
          + (np.arange(NCH * 8) // 8 * CH)[None, :]
          + outs[c]["cidx"].astype(np.int64))
         for c in range(NCORES)], axis=1)                      # [B, 1024]

    NSEL = 32
    sel = np.argsort(-vals, axis=1)[:, :NSEL]
    rows = np.arange(B)[:, None]
    sel_gidx = gidx[rows, sel]                                  # [B, NSEL]

    # exact float64 rescore of the surviving candidates
    cols = ref_queue.T[sel_gidx.reshape(-1)].reshape(B, NSEL, R)
    s64 = np.einsum("bnr,br->bn", cols.astype(np.float64),
                    ref_feats.astype(np.float64))
    # re-apply the same-id mask and kill duplicate candidates
    bad = idx_i[:, None] == iq_i[sel_gidx]
    s64[bad] = -np.inf
    order = np.argsort(-s64, axis=1, kind="stable")
    win = np.empty((B, TOPK), dtype=np.int64)
    for r in range(B):
        seen = set()
        w = []
        for j in order[r]:
            g = int(sel_gidx[r, j])
            if g not in seen and np.isfinite(s64[r, j]):
                seen.add(g)
                w.append(g)
                if len(w) == TOPK:
                    break
        win[r] = w

    score[rows, B + win] *= -1.0
    mask[rows, B + win] = 1
    return score, mask


# revision 6
# speedup vs baseline: 1.6466x; 1.6466x over previous
"""Trainium2 Bass kernel for the CoSSL retrieval/hard-negative-mining module.

Reference computation (B=256, D=128, R=2304, Q=65536, TOPK=5):
    qn = l2norm(q); kn = l2norm(k)
    score_batch = qn @ kn.T                      [B, B]
    score_queue = qn @ moco_queue                [B, Q]
    score_ref   = ref_feats @ ref_queue          [B, Q]
    mask_eq     = indices[:,None] == index_queue [B, Q]
    top5        = topk(where(mask_eq, -inf, score_ref), 5)
    score_queue = score_queue * score_ref * (+1 at top5 else -1)
    mask_queue  = mask_eq.astype(i32) with top5 set to 1
    return concat([score_batch, score_queue], 1), concat([mask_batch, mask_queue], 1)

Sharding: queues column-sharded across 8 NeuronCores (8192 cols each).
Each core computes its slice of score_queue/score_ref/mask plus the
device-local top-8 candidates per chunk of the masked score_ref
(DVE max/max_index). The host merges per-core candidates, rescores the
~32 survivors per row exactly in float64 (the distributed top-k merge),
and patches the +-1 sign / mask at the 5 winning positions per row.
The superset property (true top-5 always lands in per-chunk top-8) holds
structurally: a global top-5 element has at most 4 better elements
anywhere, so only approximation noise could push it below rank 8 in its
own chunk; measured margin is huge (worst observed in-chunk rank: 1).

REF_MODE selects the precision/speed point of the big score_ref matmul:
  "bf16": ref_queue/ref_feats streamed as bf16 (half DMA bytes, full PE
          rate). End-to-end score error ~1.3e-3 of absmax.
  "f32r": fp32 bytes streamed, PE reads them as float32r.
          End-to-end score error ~1.6e-4 of absmax.
score_queue / score_batch always run in fp32/fp32r precision.
"""

import sys

for _p in ("/opt/trn_rl_repo",):
    if _p not in sys.path:
        sys.path.insert(0, _p)

import ml_dtypes
import numpy as np

import concourse.bass as bass
import concourse.mybir as mybir
import concourse.tile as tile
from concourse import bacc
from concourse.bass_utils import run_bass_kernel_spmd
from concourse.masks import make_identity

B = 256
D = 128
R = 2304
Q = 65536
NCORES = 8
QS = Q // NCORES          # 8192 columns per core
KT = R // 128             # 18 contraction tiles
TOPK = 5
NEG_BIG = -1.0e30

F32 = mybir.dt.float32
F32R = mybir.dt.float32r
BF16 = mybir.dt.bfloat16
I32 = mybir.dt.int32
I8 = mybir.dt.int8
I16 = mybir.dt.int16
U32 = mybir.dt.uint32

REF_MODE = "bf16"         # "bf16" | "f32r"

# set True (e.g. from test.py) to capture an NTFF profile; exec time lands in
# LAST_EXEC_NS after each kernel() call.
TRACE = False
LAST_EXEC_NS = None

_CACHED = {}


def _build(mode):
    ref_dt = BF16 if mode == "bf16" else F32R
    CHD = 1024 if mode == "bf16" else 512   # DMA chunk => 2KB lines either way
    NCHD = QS // CHD
    NH = CHD // 512                          # 512-wide PSUM sub-chunks

    nc = bacc.Bacc("TRN2", target_bir_lowering=False, debug=False)

    refq_d = nc.dram_tensor("refq", [R, QS], ref_dt, kind="ExternalInput")
    moco_d = nc.dram_tensor("moco", [D, QS], F32, kind="ExternalInput")
    iq_d = nc.dram_tensor("iq", [1, QS], I16, kind="ExternalInput")
    idx_d = nc.dram_tensor("idx", [B, 1], F32, kind="ExternalInput")
    idxrow_d = nc.dram_tensor("idxrow", [1, B], F32, kind="ExternalInput")
    q_d = nc.dram_tensor("q", [B, D], F32, kind="ExternalInput")
    k_d = nc.dram_tensor("k", [B, D], F32, kind="ExternalInput")
    refT_d = nc.dram_tensor("refT", [R, B], ref_dt, kind="ExternalInput")

    prod_d = nc.dram_tensor("prod", [B, QS], F32, kind="ExternalOutput")
    maskq_d = nc.dram_tensor("maskq", [B, QS], I8, kind="ExternalOutput")
    cvals_d = nc.dram_tensor("cvals", [B, NCHD * 8], F32, kind="ExternalOutput")
    cidx_d = nc.dram_tensor("cidx", [B, NCHD * 8], U32, kind="ExternalOutput")
    sb_d = nc.dram_tensor("sb", [B, B], F32, kind="ExternalOutput")
    maskb_d = nc.dram_tensor("maskb", [B, B], I32, kind="ExternalOutput")

    with tile.TileContext(nc) as tc:
        with tc.tile_pool(name="const", bufs=1) as cpool, \
             tc.tile_pool(name="refrhs", bufs=2) as refpool, \
             tc.tile_pool(name="mocorhs", bufs=2) as mocopool, \
             tc.tile_pool(name="work", bufs=2) as wpool, \
             tc.tile_pool(name="outstage", bufs=2) as opool, \
             tc.tile_pool(name="dramscratch", bufs=1, space="DRAM") as dpool, \
             tc.tile_pool(name="psum_sr", bufs=4, space="PSUM") as srpsum, \
             tc.tile_pool(name="psum_sq", bufs=2, space="PSUM") as sqpsum, \
             tc.tile_pool(name="psum_misc", bufs=2, space="PSUM") as mpsum:

            # ---- small persistent tensors -------------------------------
            iqrow = cpool.tile([1, QS], I16, tag="iqrow")
            nc.scalar.dma_start(out=iqrow[:], in_=iq_d[:])
            iq_s = cpool.tile([128, QS], I16, tag="iq")
            nc.gpsimd.partition_broadcast(iq_s[:], iqrow[:])

            idx_s = []          # per m-tile [128,1] per-partition scalars
            for m in range(2):
                t = cpool.tile([128, 1], F32, tag=f"idx{m}", name=f"idx{m}")
                nc.scalar.dma_start(out=t[:], in_=idx_d[m * 128:(m + 1) * 128, :])
                idx_s.append(t)

            idxrow_s = cpool.tile([128, B], F32, tag="idxrow")
            nc.scalar.dma_start(out=idxrow_s[:],
                                in_=idxrow_d[:].partition_broadcast(128))

            lhsT = cpool.tile([128, KT * B], ref_dt, tag="lhsT")
            for kt in range(KT):
                nc.sync.dma_start(
                    out=lhsT[:, kt * B:(kt + 1) * B],
                    in_=refT_d[kt * 128:(kt + 1) * 128, :])

            ident = cpool.tile([128, 128], F32, tag="ident")
            make_identity(nc, ident[:])

            # ---- normalize q,k and build transposed copies --------------
            qnT = cpool.tile([128, B], F32, tag="qnT")
            knT = cpool.tile([128, B], F32, tag="knT")
            for (src_d, dstT) in ((q_d, qnT), (k_d, knT)):
                for m in range(2):
                    raw = wpool.tile([128, D], F32, tag="rawqk")
                    nc.scalar.dma_start(out=raw[:],
                                        in_=src_d[m * 128:(m + 1) * 128, :])
                    sqv = wpool.tile([128, D], F32, tag="sqv")
                    ssum = wpool.tile([128, 1], F32, tag="ssum")
                    nc.scalar.activation(
                        out=sqv[:], in_=raw[:],
                        func=mybir.ActivationFunctionType.Square,
                        accum_out=ssum[:])
                    rec = wpool.tile([128, 1], F32, tag="rec")
                    nc.vector.reciprocal(out=rec[:], in_=ssum[:])
                    inv = wpool.tile([128, 1], F32, tag="inv")
                    nc.scalar.sqrt(out=inv[:], in_=rec[:])
                    nrm = wpool.tile([128, D], F32, tag="nrm")
                    nc.vector.tensor_scalar_mul(nrm[:], raw[:], inv[:])
                    pt = mpsum.tile([128, 128], F32, tag='miscp', name='pt')
                    nc.tensor.transpose(pt[:], nrm[:], ident[:])
                    nc.scalar.copy(out=dstT[:, m * 128:(m + 1) * 128], in_=pt[:])

            # float32r view of qnT via a DRAM bounce (the walrus verifier
            # wants an f32r-typed producer for matmul operands)
            qnT_dram = dpool.tile([128, B], F32, tag="qnTd")
            nc.scalar.dma_start(out=qnT_dram[:], in_=qnT[:])
            qnT_r = cpool.tile([128, B], F32R, tag="qnTr")
            nc.scalar.dma_start(out=qnT_r[:], in_=qnT_dram[:].bitcast(F32R))

            # ---- score_batch + mask_batch -------------------------------
            for m in range(2):
                psb = mpsum.tile([128, B], F32, tag='miscp', name='psb')
                nc.tensor.matmul(psb[:], qnT[:, m * 128:(m + 1) * 128], knT[:],
                                 start=True, stop=True)
                sb_s = opool.tile([128, B], F32, tag="sb")
                nc.scalar.copy(out=sb_s[:], in_=psb[:])
                nc.scalar.dma_start(out=sb_d[m * 128:(m + 1) * 128, :], in_=sb_s[:])

                mb_s = opool.tile([128, B], I32, tag="mb")
                nc.vector.tensor_scalar(mb_s[:], idxrow_s[:], idx_s[m][:], None,
                                        op0=mybir.AluOpType.is_equal)
                nc.scalar.dma_start(out=maskb_d[m * 128:(m + 1) * 128, :],
                                    in_=mb_s[:])

            # ---- persistent accumulators --------------------------------
            cv_s = [cpool.tile([128, NCHD * 8], F32, tag=f"cv{m}",
                               name=f"cv{m}") for m in range(2)]
            ci_s = [cpool.tile([128, NCHD * 8], U32, tag=f"ci{m}",
                               name=f"ci{m}") for m in range(2)]
            mq_full = [cpool.tile([128, QS], I8, tag=f"mqf{m}",
                                  name=f"mqf{m}") for m in range(2)]

            # ---- main streaming loop ------------------------------------
            for n in range(NCHD):
                csl = slice(n * CHD, (n + 1) * CHD)

                rhs_ref = refpool.tile([128, KT * CHD], ref_dt, tag="rhsref")
                for kt in range(KT):
                    eng = nc.sync if kt % 2 == 0 else nc.gpsimd
                    eng.dma_start(
                        out=rhs_ref[:, kt * CHD:(kt + 1) * CHD],
                        in_=refq_d[kt * 128:(kt + 1) * 128, csl])
                rhs_moco = mocopool.tile([128, CHD], F32R, tag="rhsmoco")
                nc.scalar.dma_start(out=rhs_moco[:],
                                    in_=moco_d[:, csl].bitcast(F32R))

                for m in range(2):
                    msl = slice(m * 128, (m + 1) * 128)
                    masked = wpool.tile([128, CHD], F32, tag="masked")

                    for h in range(NH):
                        hsl_t = slice(h * 512, (h + 1) * 512)         # in tile
                        hsl_g = slice(n * CHD + h * 512,
                                      n * CHD + (h + 1) * 512)        # global

                        psq = sqpsum.tile([128, 512], F32)
                        nc.tensor.matmul(psq[:], qnT_r[:, msl],
                                         rhs_moco[:, hsl_t],
                                         start=True, stop=True)

                        psr = srpsum.tile([128, 512], F32)
                        for kt in range(KT):
                            nc.tensor.matmul(
                                psr[:],
                                lhsT[:, kt * B + m * 128: kt * B + (m + 1) * 128],
                                rhs_ref[:, kt * CHD + h * 512:
                                        kt * CHD + (h + 1) * 512],
                                start=(kt == 0), stop=(kt == KT - 1))

                        sq_neg = wpool.tile([128, 512], F32, tag="sqneg")
                        nc.scalar.activation(
                            out=sq_neg[:], in_=psq[:],
                            func=mybir.ActivationFunctionType.Copy,
                            scale=-1.0)

                        nc.vector.tensor_scalar(
                            mq_full[m][:, hsl_g], iq_s[:, hsl_g], idx_s[m][:],
                            None, op0=mybir.AluOpType.is_equal)

                        m_scaled = wpool.tile([128, 512], F32, tag="mscaled")
                        nc.vector.tensor_scalar(m_scaled[:], iq_s[:, hsl_g],
                                                idx_s[m][:], NEG_BIG,
                                                op0=mybir.AluOpType.is_equal,
                                                op1=mybir.AluOpType.mult)

                        nc.vector.tensor_tensor(masked[:, hsl_t], psr[:],
                                                m_scaled[:],
                                                op=mybir.AluOpType.add)

                        prod_s = opool.tile([128, 512], F32, tag="prod")
                        nc.vector.tensor_tensor(prod_s[:], psr[:], sq_neg[:],
                                                op=mybir.AluOpType.mult)
                        nc.scalar.dma_start(out=prod_d[msl, hsl_g],
                                            in_=prod_s[:])

                    nc.vector.max(out=cv_s[m][:, n * 8:(n + 1) * 8],
                                  in_=masked[:])
                    nc.vector.max_index(out=ci_s[m][:, n * 8:(n + 1) * 8],
                                        in_max=cv_s[m][:, n * 8:(n + 1) * 8],
                                        in_values=masked[:])

            for m in range(2):
                msl = slice(m * 128, (m + 1) * 128)
                nc.sync.dma_start(out=maskq_d[msl, :], in_=mq_full[m][:])
                nc.gpsimd.dma_start(out=cvals_d[msl, :], in_=cv_s[m][:])
                nc.gpsimd.dma_start(out=cidx_d[msl, :], in_=ci_s[m][:])

    nc.finalize()
    return nc, CHD, NCHD


def _get_built(mode):
    if mode not in _CACHED:
        _CACHED[mode] = _build(mode)
    return _CACHED[mode]


def kernel(q, k, ref_feats, moco_queue, ref_queue, indices, index_queue):
    global LAST_EXEC_NS
    mode = REF_MODE
    q = np.ascontiguousarray(q, dtype=np.float32)
    k = np.ascontiguousarray(k, dtype=np.float32)
    ref_feats = np.ascontiguousarray(ref_feats, dtype=np.float32)
    moco_queue = np.ascontiguousarray(moco_queue, dtype=np.float32)
    ref_queue = np.ascontiguousarray(ref_queue, dtype=np.float32)
    idx_i = np.asarray(indices)
    iq_i = np.asarray(index_queue)

    nc, CHD, NCHD = _get_built(mode)

    ref_np_dt = ml_dtypes.bfloat16 if mode == "bf16" else np.float32
    idx_f = idx_i.astype(np.float32).reshape(B, 1)
    idxrow_f = idx_i.astype(np.float32).reshape(1, B)
    refT = np.ascontiguousarray(ref_feats.T.astype(ref_np_dt))
    refq_cast = ref_queue.astype(ref_np_dt)

    in_maps = []
    for c in range(NCORES):
        sl = slice(c * QS, (c + 1) * QS)
        in_maps.append({
            "refq": np.ascontiguousarray(refq_cast[:, sl]),
            "moco": np.ascontiguousarray(moco_queue[:, sl]),
            "iq": iq_i[sl].astype(np.int16).reshape(1, QS),
            "idx": idx_f,
            "idxrow": idxrow_f,
            "q": q,
            "k": k,
            "refT": refT,
        })

    kwargs = {}
    if TRACE:
        kwargs.update(trace=True, trace_cores=list(range(NCORES)))
    res = run_bass_kernel_spmd(nc, in_maps, core_ids=list(range(NCORES)),
                               **kwargs)
    LAST_EXEC_NS = res.exec_time_ns
    outs = res.results

    score = np.empty((B, B + Q), dtype=np.float32)
    mask = np.empty((B, B + Q), dtype=np.int32)
    score[:, :B] = outs[0]["sb"]
    mask[:, :B] = outs[0]["maskb"]
    for c in range(NCORES):
        sl = slice(B + c * QS, B + (c + 1) * QS)
        score[:, sl] = outs[c]["prod"]
        mask[:, sl] = outs[c]["maskq"].astype(np.int32)

    # ---- distributed top-k merge --------------------------------------
    # candidates: per core, per CHD-chunk, top-8 (value, in-chunk index)
    vals = np.concatenate([outs[c]["cvals"] for c in range(NCORES)], axis=1)
    gidx = np.concatenate(
        [(c * QS
          + (np.arange(NCHD * 8) // 8 * CHD)[None, :]
          + outs[c]["cidx"].astype(np.int64))
         for c in range(NCORES)], axis=1)                      # [B, NCORES*NCHD*8]

    NSEL = 32
    sel = np.argsort(-vals, axis=1)[:, :NSEL]
    rows = np.arange(B)[:, None]
    sel_gidx = gidx[rows, sel]                                  # [B, NSEL]

    # exact float64 rescore of the surviving candidates
    cols = ref_queue.T[sel_gidx.reshape(-1)].reshape(B, NSEL, R)
    s64 = np.einsum("bnr,br->bn", cols.astype(np.float64),
                    ref_feats.astype(np.float64))
    # re-apply the same-id mask and kill duplicate candidates
    bad = idx_i[:, None] == iq_i[sel_gidx]
    s64[bad] = -np.inf
    order = np.argsort(-s64, axis=1, kind="stable")
    win = np.empty((B, TOPK), dtype=np.int64)
    for r in range(B):
        seen = set()
        w = []
        for j in order[r]:
            g = int(sel_gidx[r, j])
            if g not in seen and np.isfinite(s64[r, j]):
                seen.add(g)
                w.append(g)
                if len(w) == TOPK:
                    break
        win[r] = w

    score[rows, B + win] *= -1.0
    mask[rows, B + win] = 1
    return score, mask


# revision 7
# speedup vs baseline: 1.6997x; 1.0323x over previous
"""Trainium2 Bass kernel for the CoSSL retrieval/hard-negative-mining module.

Reference computation (B=256, D=128, R=2304, Q=65536, TOPK=5):
    qn = l2norm(q); kn = l2norm(k)
    score_batch = qn @ kn.T                      [B, B]
    score_queue = qn @ moco_queue                [B, Q]
    score_ref   = ref_feats @ ref_queue          [B, Q]
    mask_eq     = indices[:,None] == index_queue [B, Q]
    top5        = topk(where(mask_eq, -inf, score_ref), 5)
    score_queue = score_queue * score_ref * (+1 at top5 else -1)
    mask_queue  = mask_eq.astype(i32) with top5 set to 1
    return concat([score_batch, score_queue], 1), concat([mask_batch, mask_queue], 1)

Sharding: queues column-sharded across 8 NeuronCores (8192 cols each).
Each core computes its slice of score_queue/score_ref/mask plus the
device-local top-8 candidates per chunk of the masked score_ref
(DVE max/max_index). The host merges per-core candidates, rescores the
~32 survivors per row exactly in float64 (the distributed top-k merge),
and patches the +-1 sign / mask at the 5 winning positions per row.
The superset property (true top-5 always lands in per-chunk top-8) holds
structurally: a global top-5 element has at most 4 better elements
anywhere, so only approximation noise could push it below rank 8 in its
own chunk; measured margin is huge (worst observed in-chunk rank: 1).

REF_MODE selects the precision/speed point of the big score_ref matmul:
  "bf16": ref_queue/ref_feats streamed as bf16 (half DMA bytes, full PE
          rate). End-to-end score error ~1.3e-3 of absmax.
  "f32r": fp32 bytes streamed, PE reads them as float32r.
          End-to-end score error ~1.6e-4 of absmax.
score_queue / score_batch always run in fp32/fp32r precision.
"""

import sys

for _p in ("/opt/trn_rl_repo",):
    if _p not in sys.path:
        sys.path.insert(0, _p)

import ml_dtypes
import numpy as np

import concourse.bass as bass
import concourse.mybir as mybir
import concourse.tile as tile
from concourse import bacc
from concourse.bass_utils import run_bass_kernel_spmd
from concourse.masks import make_identity

B = 256
D = 128
R = 2304
Q = 65536
NCORES = 8
QS = Q // NCORES          # 8192 columns per core
KT = R // 128             # 18 contraction tiles
TOPK = 5
NEG_BIG = -1.0e30

F32 = mybir.dt.float32
F32R = mybir.dt.float32r
BF16 = mybir.dt.bfloat16
I32 = mybir.dt.int32
I8 = mybir.dt.int8
I16 = mybir.dt.int16
U32 = mybir.dt.uint32

REF_MODE = "bf16"         # "bf16" | "f32r"

# set True (e.g. from test.py) to capture an NTFF profile; exec time lands in
# LAST_EXEC_NS after each kernel() call.
TRACE = False
LAST_EXEC_NS = None

_CACHED = {}


def _build(mode):
    ref_dt = BF16 if mode == "bf16" else F32R
    CHD = 1024 if mode == "bf16" else 512   # DMA chunk => 2KB lines either way
    NCHD = QS // CHD
    NH = CHD // 512                          # 512-wide PSUM sub-chunks

    nc = bacc.Bacc("TRN2", target_bir_lowering=False, debug=False)

    refq_d = nc.dram_tensor("refq", [R, QS], ref_dt, kind="ExternalInput")
    moco_d = nc.dram_tensor("moco", [D, QS], F32, kind="ExternalInput")
    iq_d = nc.dram_tensor("iq", [1, QS], I16, kind="ExternalInput")
    idx_d = nc.dram_tensor("idx", [B, 1], F32, kind="ExternalInput")
    idxrow_d = nc.dram_tensor("idxrow", [1, B], F32, kind="ExternalInput")
    q_d = nc.dram_tensor("q", [B, D], F32, kind="ExternalInput")
    k_d = nc.dram_tensor("k", [B, D], F32, kind="ExternalInput")
    refT_d = nc.dram_tensor("refT", [R, B], ref_dt, kind="ExternalInput")

    prod_d = nc.dram_tensor("prod", [B, QS], F32, kind="ExternalOutput")
    maskq_d = nc.dram_tensor("maskq", [B, QS], I8, kind="ExternalOutput")
    cvals_d = nc.dram_tensor("cvals", [B, NCHD * NH * 8], F32, kind="ExternalOutput")
    cidx_d = nc.dram_tensor("cidx", [B, NCHD * NH * 8], U32, kind="ExternalOutput")
    sb_d = nc.dram_tensor("sb", [B, B], F32, kind="ExternalOutput")
    maskb_d = nc.dram_tensor("maskb", [B, B], I32, kind="ExternalOutput")

    with tile.TileContext(nc) as tc:
        with tc.tile_pool(name="const", bufs=1) as cpool, \
             tc.tile_pool(name="refrhs", bufs=2) as refpool, \
             tc.tile_pool(name="mocorhs", bufs=2) as mocopool, \
             tc.tile_pool(name="work", bufs=2) as wpool, \
             tc.tile_pool(name="outstage", bufs=2) as opool, \
             tc.tile_pool(name="dramscratch", bufs=1, space="DRAM") as dpool, \
             tc.tile_pool(name="psum_sr", bufs=4, space="PSUM") as srpsum, \
             tc.tile_pool(name="psum_sq", bufs=2, space="PSUM") as sqpsum, \
             tc.tile_pool(name="psum_misc", bufs=2, space="PSUM") as mpsum:

            # ---- small persistent tensors -------------------------------
            iq_s = cpool.tile([128, QS], I16, tag="iq")
            nc.gpsimd.dma_start(out=iq_s[:], in_=iq_d[:].partition_broadcast(128))

            idx_s = []          # per m-tile [128,1] per-partition scalars
            for m in range(2):
                t = cpool.tile([128, 1], F32, tag=f"idx{m}", name=f"idx{m}")
                nc.scalar.dma_start(out=t[:], in_=idx_d[m * 128:(m + 1) * 128, :])
                idx_s.append(t)

            idxrow_s = cpool.tile([128, B], F32, tag="idxrow")
            nc.scalar.dma_start(out=idxrow_s[:],
                                in_=idxrow_d[:].partition_broadcast(128))

            lhsT = cpool.tile([128, KT * B], ref_dt, tag="lhsT")
            nc.scalar.dma_start(
                out=lhsT[:],
                in_=bass.AP(refT_d, 0, [[B, 128], [128 * B, KT], [1, B]]))

            ident = cpool.tile([128, 128], F32, tag="ident")
            make_identity(nc, ident[:])

            # ---- normalize q,k and build transposed copies --------------
            qnT = cpool.tile([128, B], F32, tag="qnT")
            knT = cpool.tile([128, B], F32, tag="knT")
            for (src_d, dstT) in ((q_d, qnT), (k_d, knT)):
                for m in range(2):
                    raw = wpool.tile([128, D], F32, tag="rawqk")
                    nc.scalar.dma_start(out=raw[:],
                                        in_=src_d[m * 128:(m + 1) * 128, :])
                    sqv = wpool.tile([128, D], F32, tag="sqv")
                    ssum = wpool.tile([128, 1], F32, tag="ssum")
                    nc.scalar.activation(
                        out=sqv[:], in_=raw[:],
                        func=mybir.ActivationFunctionType.Square,
                        accum_out=ssum[:])
                    rec = wpool.tile([128, 1], F32, tag="rec")
                    nc.vector.reciprocal(out=rec[:], in_=ssum[:])
                    inv = wpool.tile([128, 1], F32, tag="inv")
                    nc.scalar.sqrt(out=inv[:], in_=rec[:])
                    nrm = wpool.tile([128, D], F32, tag="nrm")
                    nc.vector.tensor_scalar_mul(nrm[:], raw[:], inv[:])
                    pt = mpsum.tile([128, 128], F32, tag='miscp', name='pt')
                    nc.tensor.transpose(pt[:], nrm[:], ident[:])
                    nc.scalar.copy(out=dstT[:, m * 128:(m + 1) * 128], in_=pt[:])

            # float32r view of qnT via a DRAM bounce (the walrus verifier
            # wants an f32r-typed producer for matmul operands)
            qnT_dram = dpool.tile([128, B], F32, tag="qnTd")
            nc.scalar.dma_start(out=qnT_dram[:], in_=qnT[:])
            qnT_r = cpool.tile([128, B], F32R, tag="qnTr")
            nc.scalar.dma_start(out=qnT_r[:], in_=qnT_dram[:].bitcast(F32R))

            # ---- score_batch + mask_batch -------------------------------
            for m in range(2):
                psb = mpsum.tile([128, B], F32, tag='miscp', name='psb')
                nc.tensor.matmul(psb[:], qnT[:, m * 128:(m + 1) * 128], knT[:],
                                 start=True, stop=True)
                sb_s = opool.tile([128, B], F32, tag="sb")
                nc.scalar.copy(out=sb_s[:], in_=psb[:])
                nc.scalar.dma_start(out=sb_d[m * 128:(m + 1) * 128, :], in_=sb_s[:])

                mb_s = opool.tile([128, B], I32, tag="mb")
                nc.vector.tensor_scalar(mb_s[:], idxrow_s[:], idx_s[m][:], None,
                                        op0=mybir.AluOpType.is_equal)
                nc.scalar.dma_start(out=maskb_d[m * 128:(m + 1) * 128, :],
                                    in_=mb_s[:])

            # ---- persistent accumulators --------------------------------
            cv_s = [cpool.tile([128, NCHD * NH * 8], F32, tag=f"cv{m}",
                               name=f"cv{m}") for m in range(2)]
            ci_s = [cpool.tile([128, NCHD * NH * 8], U32, tag=f"ci{m}",
                               name=f"ci{m}") for m in range(2)]
            mq_full = [cpool.tile([128, QS], I8, tag=f"mqf{m}",
                                  name=f"mqf{m}") for m in range(2)]

            # ---- main streaming loop ------------------------------------
            for n in range(NCHD):
                csl = slice(n * CHD, (n + 1) * CHD)

                rhs_ref = refpool.tile([128, KT * CHD], ref_dt, tag="rhsref")
                eng = nc.sync if n % 2 == 0 else nc.gpsimd
                eng.dma_start(
                    out=rhs_ref[:],
                    in_=bass.AP(refq_d, n * CHD,
                                [[QS, 128], [128 * QS, KT], [1, CHD]]))
                rhs_moco = mocopool.tile([128, CHD], F32R, tag="rhsmoco")
                nc.scalar.dma_start(out=rhs_moco[:],
                                    in_=moco_d[:, csl].bitcast(F32R))

                for m in range(2):
                    msl = slice(m * 128, (m + 1) * 128)
                    nc.vector.tensor_scalar(
                        mq_full[m][:, csl], iq_s[:, csl], idx_s[m][:],
                        None, op0=mybir.AluOpType.is_equal)

                    for h in range(NH):
                        hsl_t = slice(h * 512, (h + 1) * 512)         # in tile
                        hsl_g = slice(n * CHD + h * 512,
                                      n * CHD + (h + 1) * 512)        # global

                        psq = sqpsum.tile([128, 512], F32)
                        nc.tensor.matmul(psq[:], qnT_r[:, msl],
                                         rhs_moco[:, hsl_t],
                                         start=True, stop=True)

                        psr = srpsum.tile([128, 512], F32)
                        for kt in range(KT):
                            nc.tensor.matmul(
                                psr[:],
                                lhsT[:, kt * B + m * 128: kt * B + (m + 1) * 128],
                                rhs_ref[:, kt * CHD + h * 512:
                                        kt * CHD + (h + 1) * 512],
                                start=(kt == 0), stop=(kt == KT - 1))

                        sq_neg = wpool.tile([128, 512], F32, tag="sqneg")
                        nc.scalar.activation(
                            out=sq_neg[:], in_=psq[:],
                            func=mybir.ActivationFunctionType.Copy,
                            scale=-1.0)

                        prod_s = opool.tile([128, 512], F32, tag="prod")
                        nc.vector.tensor_tensor(prod_s[:], psr[:], sq_neg[:],
                                                op=mybir.AluOpType.mult)
                        nc.scalar.dma_start(out=prod_d[msl, hsl_g],
                                            in_=prod_s[:])

                        sl8 = slice((n * NH + h) * 8, (n * NH + h + 1) * 8)
                        nc.vector.max(out=cv_s[m][:, sl8], in_=psr[:])
                        nc.vector.max_index(out=ci_s[m][:, sl8],
                                            in_max=cv_s[m][:, sl8],
                                            in_values=psr[:])

            for m in range(2):
                msl = slice(m * 128, (m + 1) * 128)
                nc.sync.dma_start(out=maskq_d[msl, :], in_=mq_full[m][:])
                nc.gpsimd.dma_start(out=cvals_d[msl, :], in_=cv_s[m][:])
                nc.gpsimd.dma_start(out=cidx_d[msl, :], in_=ci_s[m][:])

    nc.finalize()
    return nc, CHD, NCHD


def _get_built(mode):
    if mode not in _CACHED:
        _CACHED[mode] = _build(mode)
    return _CACHED[mode]


def kernel(q, k, ref_feats, moco_queue, ref_queue, indices, index_queue):
    global LAST_EXEC_NS
    mode = REF_MODE
    q = np.ascontiguousarray(q, dtype=np.float32)
    k = np.ascontiguousarray(k, dtype=np.float32)
    ref_feats = np.ascontiguousarray(ref_feats, dtype=np.float32)
    moco_queue = np.ascontiguousarray(moco_queue, dtype=np.float32)
    ref_queue = np.ascontiguousarray(ref_queue, dtype=np.float32)
    idx_i = np.asarray(indices)
    iq_i = np.asarray(index_queue)

    nc, CHD, NCHD = _get_built(mode)

    ref_np_dt = ml_dtypes.bfloat16 if mode == "bf16" else np.float32
    idx_f = idx_i.astype(np.float32).reshape(B, 1)
    idxrow_f = idx_i.astype(np.float32).reshape(1, B)
    refT = np.ascontiguousarray(ref_feats.T.astype(ref_np_dt))
    refq_cast = ref_queue.astype(ref_np_dt)

    in_maps = []
    for c in range(NCORES):
        sl = slice(c * QS, (c + 1) * QS)
        in_maps.append({
            "refq": np.ascontiguousarray(refq_cast[:, sl]),
            "moco": np.ascontiguousarray(moco_queue[:, sl]),
            "iq": iq_i[sl].astype(np.int16).reshape(1, QS),
            "idx": idx_f,
            "idxrow": idxrow_f,
            "q": q,
            "k": k,
            "refT": refT,
        })

    kwargs = {}
    if TRACE:
        kwargs.update(trace=True, trace_cores=list(range(NCORES)))
    res = run_bass_kernel_spmd(nc, in_maps, core_ids=list(range(NCORES)),
                               **kwargs)
    LAST_EXEC_NS = res.exec_time_ns
    outs = res.results

    score = np.empty((B, B + Q), dtype=np.float32)
    mask = np.empty((B, B + Q), dtype=np.int32)
    score[:, :B] = outs[0]["sb"]
    mask[:, :B] = outs[0]["maskb"]
    for c in range(NCORES):
        sl = slice(B + c * QS, B + (c + 1) * QS)
        score[:, sl] = outs[c]["prod"]
        mask[:, sl] = outs[c]["maskq"].astype(np.int32)

    # ---- distributed top-k merge --------------------------------------
    # candidates: per core, per CHD-chunk, top-8 (value, in-chunk index)
    vals = np.concatenate([outs[c]["cvals"] for c in range(NCORES)], axis=1)
    ncand = vals.shape[1] // NCORES
    gidx = np.concatenate(
        [(c * QS
          + (np.arange(ncand) // 8 * 512)[None, :]
          + outs[c]["cidx"].astype(np.int64))
         for c in range(NCORES)], axis=1)

    NSEL = 32
    sel = np.argsort(-vals, axis=1)[:, :NSEL]
    rows = np.arange(B)[:, None]
    sel_gidx = gidx[rows, sel]                                  # [B, NSEL]

    # exact float64 rescore of the surviving candidates
    cols = ref_queue.T[sel_gidx.reshape(-1)].reshape(B, NSEL, R)
    s64 = np.einsum("bnr,br->bn", cols.astype(np.float64),
                    ref_feats.astype(np.float64))
    # re-apply the same-id mask and kill duplicate candidates
    bad = idx_i[:, None] == iq_i[sel_gidx]
    s64[bad] = -np.inf
    order = np.argsort(-s64, axis=1, kind="stable")
    win = np.empty((B, TOPK), dtype=np.int64)
    for r in range(B):
        seen = set()
        w = []
        for j in order[r]:
            g = int(sel_gidx[r, j])
            if g not in seen and np.isfinite(s64[r, j]):
                seen.add(g)
                w.append(g)
                if len(w) == TOPK:
                    break
        win[r] = w

    score[rows, B + win] *= -1.0
    mask[rows, B + win] = 1
    return score, mask


# revision 8
# speedup vs baseline: 1.7460x; 1.0272x over previous
"""Trainium2 Bass kernel for the CoSSL retrieval/hard-negative-mining module.

Reference computation (B=256, D=128, R=2304, Q=65536, TOPK=5):
    qn = l2norm(q); kn = l2norm(k)
    score_batch = qn @ kn.T                      [B, B]
    score_queue = qn @ moco_queue                [B, Q]
    score_ref   = ref_feats @ ref_queue          [B, Q]
    mask_eq     = indices[:,None] == index_queue [B, Q]
    top5        = topk(where(mask_eq, -inf, score_ref), 5)
    score_queue = score_queue * score_ref * (+1 at top5 else -1)
    mask_queue  = mask_eq.astype(i32) with top5 set to 1
    return concat([score_batch, score_queue], 1), concat([mask_batch, mask_queue], 1)

Sharding: queues column-sharded across 8 NeuronCores (8192 cols each).
Each core computes its slice of score_queue/score_ref/mask plus the
device-local top-8 candidates per chunk of the masked score_ref
(DVE max/max_index). The host merges per-core candidates, rescores the
~32 survivors per row exactly in float64 (the distributed top-k merge),
and patches the +-1 sign / mask at the 5 winning positions per row.
The superset property (true top-5 always lands in per-chunk top-8) holds
structurally: a global top-5 element has at most 4 better elements
anywhere, so only approximation noise could push it below rank 8 in its
own chunk; measured margin is huge (worst observed in-chunk rank: 1).

REF_MODE selects the precision/speed point of the big score_ref matmul:
  "bf16": ref_queue/ref_feats streamed as bf16 (half DMA bytes, full PE
          rate). End-to-end score error ~1.3e-3 of absmax.
  "f32r": fp32 bytes streamed, PE reads them as float32r.
          End-to-end score error ~1.6e-4 of absmax.
score_queue / score_batch always run in fp32/fp32r precision.
"""

import sys

for _p in ("/opt/trn_rl_repo",):
    if _p not in sys.path:
        sys.path.insert(0, _p)

import ml_dtypes
import numpy as np

import concourse.bass as bass
import concourse.mybir as mybir
import concourse.tile as tile
from concourse import bacc
from concourse.bass_utils import run_bass_kernel_spmd
from concourse.masks import make_identity

B = 256
D = 128
R = 2304
Q = 65536
NCORES = 8
QS = Q // NCORES          # 8192 columns per core
KT = R // 128             # 18 contraction tiles
TOPK = 5
NEG_BIG = -1.0e30

F32 = mybir.dt.float32
F32R = mybir.dt.float32r
BF16 = mybir.dt.bfloat16
I32 = mybir.dt.int32
I8 = mybir.dt.int8
I16 = mybir.dt.int16
U32 = mybir.dt.uint32

REF_MODE = "bf16"         # "bf16" | "f32r"

# set True (e.g. from test.py) to capture an NTFF profile; exec time lands in
# LAST_EXEC_NS after each kernel() call.
TRACE = False
LAST_EXEC_NS = None

_CACHED = {}


def _build(mode):
    ref_dt = BF16 if mode == "bf16" else F32R
    CHD = 1024 if mode == "bf16" else 512   # DMA chunk => 2KB lines either way
    NCHD = QS // CHD
    NH = CHD // 512                          # 512-wide PSUM sub-chunks

    nc = bacc.Bacc("TRN2", target_bir_lowering=False, debug=False)

    refq_d = nc.dram_tensor("refq", [R, QS], ref_dt, kind="ExternalInput")
    moco_d = nc.dram_tensor("moco", [D, QS], F32, kind="ExternalInput")
    iq_d = nc.dram_tensor("iq", [1, QS], I16, kind="ExternalInput")
    idx_d = nc.dram_tensor("idx", [B, 1], F32, kind="ExternalInput")
    idxrow_d = nc.dram_tensor("idxrow", [1, B], F32, kind="ExternalInput")
    q_d = nc.dram_tensor("q", [B, D], F32, kind="ExternalInput")
    k_d = nc.dram_tensor("k", [B, D], F32, kind="ExternalInput")
    refT_d = nc.dram_tensor("refT", [128, KT * B], ref_dt, kind="ExternalInput")

    prod_d = nc.dram_tensor("prod", [B, QS], F32, kind="ExternalOutput")
    maskq_d = nc.dram_tensor("maskq", [B, QS], I8, kind="ExternalOutput")
    cvals_d = nc.dram_tensor("cvals", [B, NCHD * NH * 8], F32, kind="ExternalOutput")
    cidx_d = nc.dram_tensor("cidx", [B, NCHD * NH * 8], U32, kind="ExternalOutput")
    sb_d = nc.dram_tensor("sb", [B, B], F32, kind="ExternalOutput")
    maskb_d = nc.dram_tensor("maskb", [B, B], I32, kind="ExternalOutput")

    with tile.TileContext(nc) as tc:
        with tc.tile_pool(name="const", bufs=1) as cpool, \
             tc.tile_pool(name="refrhs", bufs=3) as refpool, \
             tc.tile_pool(name="mocorhs", bufs=2) as mocopool, \
             tc.tile_pool(name="work", bufs=2) as wpool, \
             tc.tile_pool(name="outstage", bufs=2) as opool, \
             tc.tile_pool(name="dramscratch", bufs=1, space="DRAM") as dpool, \
             tc.tile_pool(name="psum_sr", bufs=4, space="PSUM") as srpsum, \
             tc.tile_pool(name="psum_sq", bufs=2, space="PSUM") as sqpsum, \
             tc.tile_pool(name="psum_misc", bufs=2, space="PSUM") as mpsum:

            # ---- small persistent tensors -------------------------------
            iq_s = cpool.tile([128, QS], I16, tag="iq")
            nc.gpsimd.dma_start(out=iq_s[:], in_=iq_d[:].partition_broadcast(128))

            idx_s = []          # per m-tile [128,1] per-partition scalars
            for m in range(2):
                t = cpool.tile([128, 1], F32, tag=f"idx{m}", name=f"idx{m}")
                nc.scalar.dma_start(out=t[:], in_=idx_d[m * 128:(m + 1) * 128, :])
                idx_s.append(t)

            idxrow_s = cpool.tile([128, B], F32, tag="idxrow")
            nc.scalar.dma_start(out=idxrow_s[:],
                                in_=idxrow_d[:].partition_broadcast(128))

            lhsT = cpool.tile([128, KT * B], ref_dt, tag="lhsT")
            third = KT * B // 3
            for e_i, eng in enumerate((nc.sync, nc.gpsimd, nc.scalar)):
                eng.dma_start(out=lhsT[:, e_i * third:(e_i + 1) * third],
                              in_=refT_d[:, e_i * third:(e_i + 1) * third])

            ident = cpool.tile([128, 128], F32, tag="ident")
            make_identity(nc, ident[:])

            # ---- normalize q,k and build transposed copies --------------
            qnT = cpool.tile([128, B], F32, tag="qnT")
            knT = cpool.tile([128, B], F32, tag="knT")
            for (src_d, dstT) in ((q_d, qnT), (k_d, knT)):
                for m in range(2):
                    raw = wpool.tile([128, D], F32, tag="rawqk")
                    nc.scalar.dma_start(out=raw[:],
                                        in_=src_d[m * 128:(m + 1) * 128, :])
                    sqv = wpool.tile([128, D], F32, tag="sqv")
                    ssum = wpool.tile([128, 1], F32, tag="ssum")
                    nc.scalar.activation(
                        out=sqv[:], in_=raw[:],
                        func=mybir.ActivationFunctionType.Square,
                        accum_out=ssum[:])
                    rec = wpool.tile([128, 1], F32, tag="rec")
                    nc.vector.reciprocal(out=rec[:], in_=ssum[:])
                    inv = wpool.tile([128, 1], F32, tag="inv")
                    nc.scalar.sqrt(out=inv[:], in_=rec[:])
                    nrm = wpool.tile([128, D], F32, tag="nrm")
                    nc.vector.tensor_scalar_mul(nrm[:], raw[:], inv[:])
                    pt = mpsum.tile([128, 128], F32, tag='miscp', name='pt')
                    nc.tensor.transpose(pt[:], nrm[:], ident[:])
                    nc.scalar.copy(out=dstT[:, m * 128:(m + 1) * 128], in_=pt[:])

            # float32r view of qnT via a DRAM bounce (the walrus verifier
            # wants an f32r-typed producer for matmul operands)
            qnT_dram = dpool.tile([128, B], F32, tag="qnTd")
            nc.scalar.dma_start(out=qnT_dram[:], in_=qnT[:])
            qnT_r = cpool.tile([128, B], F32R, tag="qnTr")
            nc.scalar.dma_start(out=qnT_r[:], in_=qnT_dram[:].bitcast(F32R))

            # ---- score_batch + mask_batch -------------------------------
            for m in range(2):
                psb = mpsum.tile([128, B], F32, tag='miscp', name='psb')
                nc.tensor.matmul(psb[:], qnT[:, m * 128:(m + 1) * 128], knT[:],
                                 start=True, stop=True)
                sb_s = opool.tile([128, B], F32, tag="sb")
                nc.scalar.copy(out=sb_s[:], in_=psb[:])
                nc.scalar.dma_start(out=sb_d[m * 128:(m + 1) * 128, :], in_=sb_s[:])

                mb_s = opool.tile([128, B], I32, tag="mb")
                nc.vector.tensor_scalar(mb_s[:], idxrow_s[:], idx_s[m][:], None,
                                        op0=mybir.AluOpType.is_equal)
                nc.scalar.dma_start(out=maskb_d[m * 128:(m + 1) * 128, :],
                                    in_=mb_s[:])

            # ---- persistent accumulators --------------------------------
            cv_s = [cpool.tile([128, NCHD * NH * 8], F32, tag=f"cv{m}",
                               name=f"cv{m}") for m in range(2)]
            ci_s = [cpool.tile([128, NCHD * NH * 8], U32, tag=f"ci{m}",
                               name=f"ci{m}") for m in range(2)]
            mq_full = [cpool.tile([128, QS], I8, tag=f"mqf{m}",
                                  name=f"mqf{m}") for m in range(2)]

            # ---- main streaming loop ------------------------------------
            for n in range(NCHD):
                csl = slice(n * CHD, (n + 1) * CHD)

                rhs_ref = refpool.tile([128, KT * CHD], ref_dt, tag="rhsref")
                KH = KT // 2
                for e_i, eng in enumerate((nc.sync, nc.gpsimd)):
                    kt0 = e_i * KH
                    nkt = KH if e_i == 0 else KT - KH
                    eng.dma_start(
                        out=rhs_ref[:, kt0 * CHD:(kt0 + nkt) * CHD],
                        in_=bass.AP(refq_d, kt0 * 128 * QS + n * CHD,
                                    [[QS, 128], [128 * QS, nkt], [1, CHD]]))
                rhs_moco = mocopool.tile([128, CHD], F32R, tag="rhsmoco")
                nc.scalar.dma_start(out=rhs_moco[:],
                                    in_=moco_d[:, csl].bitcast(F32R))

                for m in range(2):
                    msl = slice(m * 128, (m + 1) * 128)
                    nc.vector.tensor_scalar(
                        mq_full[m][:, csl], iq_s[:, csl], idx_s[m][:],
                        None, op0=mybir.AluOpType.is_equal)

                    for h in range(NH):
                        hsl_t = slice(h * 512, (h + 1) * 512)         # in tile
                        hsl_g = slice(n * CHD + h * 512,
                                      n * CHD + (h + 1) * 512)        # global

                        psq = sqpsum.tile([128, 512], F32)
                        nc.tensor.matmul(psq[:], qnT_r[:, msl],
                                         rhs_moco[:, hsl_t],
                                         start=True, stop=True)

                        psr = srpsum.tile([128, 512], F32)
                        for kt in range(KT):
                            nc.tensor.matmul(
                                psr[:],
                                lhsT[:, kt * B + m * 128: kt * B + (m + 1) * 128],
                                rhs_ref[:, kt * CHD + h * 512:
                                        kt * CHD + (h + 1) * 512],
                                start=(kt == 0), stop=(kt == KT - 1))

                        sq_neg = wpool.tile([128, 512], F32, tag="sqneg")
                        nc.scalar.activation(
                            out=sq_neg[:], in_=psq[:],
                            func=mybir.ActivationFunctionType.Copy,
                            scale=-1.0)

                        prod_s = opool.tile([128, 512], F32, tag="prod")
                        nc.vector.tensor_tensor(prod_s[:], psr[:], sq_neg[:],
                                                op=mybir.AluOpType.mult)
                        nc.sync.dma_start(out=prod_d[msl, hsl_g],
                                            in_=prod_s[:])

                        sl8 = slice((n * NH + h) * 8, (n * NH + h + 1) * 8)
                        nc.vector.max(out=cv_s[m][:, sl8], in_=psr[:])
                        nc.vector.max_index(out=ci_s[m][:, sl8],
                                            in_max=cv_s[m][:, sl8],
                                            in_values=psr[:])

            for m in range(2):
                msl = slice(m * 128, (m + 1) * 128)
                nc.sync.dma_start(out=maskq_d[msl, :], in_=mq_full[m][:])
                nc.gpsimd.dma_start(out=cvals_d[msl, :], in_=cv_s[m][:])
                nc.gpsimd.dma_start(out=cidx_d[msl, :], in_=ci_s[m][:])

    nc.finalize()
    return nc, CHD, NCHD


def _get_built(mode):
    if mode not in _CACHED:
        _CACHED[mode] = _build(mode)
    return _CACHED[mode]


def kernel(q, k, ref_feats, moco_queue, ref_queue, indices, index_queue):
    global LAST_EXEC_NS
    mode = REF_MODE
    q = np.ascontiguousarray(q, dtype=np.float32)
    k = np.ascontiguousarray(k, dtype=np.float32)
    ref_feats = np.ascontiguousarray(ref_feats, dtype=np.float32)
    moco_queue = np.ascontiguousarray(moco_queue, dtype=np.float32)
    ref_queue = np.ascontiguousarray(ref_queue, dtype=np.float32)
    idx_i = np.asarray(indices)
    iq_i = np.asarray(index_queue)

    nc, CHD, NCHD = _get_built(mode)

    ref_np_dt = ml_dtypes.bfloat16 if mode == "bf16" else np.float32
    idx_f = idx_i.astype(np.float32).reshape(B, 1)
    idxrow_f = idx_i.astype(np.float32).reshape(1, B)
    refT = np.ascontiguousarray(
        ref_feats.T.astype(ref_np_dt).reshape(KT, 128, B)
        .transpose(1, 0, 2).reshape(128, KT * B))
    refq_cast = ref_queue.astype(ref_np_dt)

    in_maps = []
    for c in range(NCORES):
        sl = slice(c * QS, (c + 1) * QS)
        in_maps.append({
            "refq": np.ascontiguousarray(refq_cast[:, sl]),
            "moco": np.ascontiguousarray(moco_queue[:, sl]),
            "iq": iq_i[sl].astype(np.int16).reshape(1, QS),
            "idx": idx_f,
            "idxrow": idxrow_f,
            "q": q,
            "k": k,
            "refT": refT,
        })

    kwargs = {}
    if TRACE:
        kwargs.update(trace=True, trace_cores=list(range(NCORES)))
    res = run_bass_kernel_spmd(nc, in_maps, core_ids=list(range(NCORES)),
                               **kwargs)
    LAST_EXEC_NS = res.exec_time_ns
    outs = res.results

    score = np.empty((B, B + Q), dtype=np.float32)
    mask = np.empty((B, B + Q), dtype=np.int32)
    score[:, :B] = outs[0]["sb"]
    mask[:, :B] = outs[0]["maskb"]
    for c in range(NCORES):
        sl = slice(B + c * QS, B + (c + 1) * QS)
        score[:, sl] = outs[c]["prod"]
        mask[:, sl] = outs[c]["maskq"].astype(np.int32)

    # ---- distributed top-k merge --------------------------------------
    # candidates: per core, per CHD-chunk, top-8 (value, in-chunk index)
    vals = np.concatenate([outs[c]["cvals"] for c in range(NCORES)], axis=1)
    ncand = vals.shape[1] // NCORES
    gidx = np.concatenate(
        [(c * QS
          + (np.arange(ncand) // 8 * 512)[None, :]
          + outs[c]["cidx"].astype(np.int64))
         for c in range(NCORES)], axis=1)

    NSEL = 32
    sel = np.argsort(-vals, axis=1)[:, :NSEL]
    rows = np.arange(B)[:, None]
    sel_gidx = gidx[rows, sel]                                  # [B, NSEL]

    # exact float64 rescore of the surviving candidates
    cols = ref_queue.T[sel_gidx.reshape(-1)].reshape(B, NSEL, R)
    s64 = np.einsum("bnr,br->bn", cols.astype(np.float64),
                    ref_feats.astype(np.float64))
    # re-apply the same-id mask and kill duplicate candidates
    bad = idx_i[:, None] == iq_i[sel_gidx]
    s64[bad] = -np.inf
    order = np.argsort(-s64, axis=1, kind="stable")
    win = np.empty((B, TOPK), dtype=np.int64)
    for r in range(B):
        seen = set()
        w = []
        for j in order[r]:
            g = int(sel_gidx[r, j])
            if g not in seen and np.isfinite(s64[r, j]):
                seen.add(g)
                w.append(g)
                if len(w) == TOPK:
                    break
        win[r] = w

    score[rows, B + win] *= -1.0
    mask[rows, B + win] = 1
    return score, mask


# revision 9
# speedup vs baseline: 1.7569x; 1.0062x over previous
"""Trainium2 Bass kernel for the CoSSL retrieval/hard-negative-mining module.

Reference computation (B=256, D=128, R=2304, Q=65536, TOPK=5):
    qn = l2norm(q); kn = l2norm(k)
    score_batch = qn @ kn.T                      [B, B]
    score_queue = qn @ moco_queue                [B, Q]
    score_ref   = ref_feats @ ref_queue          [B, Q]
    mask_eq     = indices[:,None] == index_queue [B, Q]
    top5        = topk(where(mask_eq, -inf, score_ref), 5)
    score_queue = score_queue * score_ref * (+1 at top5 else -1)
    mask_queue  = mask_eq.astype(i32) with top5 set to 1
    return concat([score_batch, score_queue], 1), concat([mask_batch, mask_queue], 1)

Sharding: queues column-sharded across 8 NeuronCores (8192 cols each).
Each core computes its slice of score_queue/score_ref/mask plus the
device-local top-8 candidates per chunk of the masked score_ref
(DVE max/max_index). The host merges per-core candidates, rescores the
~32 survivors per row exactly in float64 (the distributed top-k merge),
and patches the +-1 sign / mask at the 5 winning positions per row.
The superset property (true top-5 always lands in per-chunk top-8) holds
structurally: a global top-5 element has at most 4 better elements
anywhere, so only approximation noise could push it below rank 8 in its
own chunk; measured margin is huge (worst observed in-chunk rank: 1).

REF_MODE selects the precision/speed point of the big score_ref matmul:
  "bf16": ref_queue/ref_feats streamed as bf16 (half DMA bytes, full PE
          rate). End-to-end score error ~1.3e-3 of absmax.
  "f32r": fp32 bytes streamed, PE reads them as float32r.
          End-to-end score error ~1.6e-4 of absmax.
score_queue / score_batch always run in fp32/fp32r precision.
"""

import sys

for _p in ("/opt/trn_rl_repo",):
    if _p not in sys.path:
        sys.path.insert(0, _p)

import ml_dtypes
import numpy as np

import concourse.bass as bass
import concourse.mybir as mybir
import concourse.tile as tile
from concourse import bacc
from concourse.bass_utils import run_bass_kernel_spmd
from concourse.masks import make_identity

B = 256
D = 128
R = 2304
Q = 65536
NCORES = 8
QS = Q // NCORES          # 8192 columns per core
KT = R // 128             # 18 contraction tiles
TOPK = 5
NEG_BIG = -1.0e30

F32 = mybir.dt.float32
F32R = mybir.dt.float32r
BF16 = mybir.dt.bfloat16
I32 = mybir.dt.int32
I8 = mybir.dt.int8
I16 = mybir.dt.int16
U32 = mybir.dt.uint32

REF_MODE = "bf16"         # "bf16" | "f32r"

# set True (e.g. from test.py) to capture an NTFF profile; exec time lands in
# LAST_EXEC_NS after each kernel() call.
TRACE = False
LAST_EXEC_NS = None

_CACHED = {}


def _build(mode):
    ref_dt = BF16 if mode == "bf16" else F32R
    CHD = 1024 if mode == "bf16" else 512   # DMA chunk => 2KB lines either way
    NCHD = QS // CHD
    NH = CHD // 512                          # 512-wide PSUM sub-chunks

    nc = bacc.Bacc("TRN2", target_bir_lowering=False, debug=False)

    refq_d = nc.dram_tensor("refq", [R, QS], ref_dt, kind="ExternalInput")
    moco_d = nc.dram_tensor("moco", [D, QS], F32, kind="ExternalInput")
    iq_d = nc.dram_tensor("iq", [1, QS], I16, kind="ExternalInput")
    idx_d = nc.dram_tensor("idx", [B, 1], F32, kind="ExternalInput")
    idxrow_d = nc.dram_tensor("idxrow", [1, B], F32, kind="ExternalInput")
    q_d = nc.dram_tensor("q", [B, D], F32, kind="ExternalInput")
    k_d = nc.dram_tensor("k", [B, D], F32, kind="ExternalInput")
    refT_d = nc.dram_tensor("refT", [128, KT * B], ref_dt, kind="ExternalInput")

    prod_d = nc.dram_tensor("prod", [B, QS], F32, kind="ExternalOutput")
    maskq_d = nc.dram_tensor("maskq", [B, QS], I8, kind="ExternalOutput")
    cvals_d = nc.dram_tensor("cvals", [B, NCHD * NH * 8], F32, kind="ExternalOutput")
    cidx_d = nc.dram_tensor("cidx", [B, NCHD * NH * 8], U32, kind="ExternalOutput")
    sb_d = nc.dram_tensor("sb", [B, B], F32, kind="ExternalOutput")
    maskb_d = nc.dram_tensor("maskb", [B, B], I32, kind="ExternalOutput")

    with tile.TileContext(nc) as tc:
        with tc.tile_pool(name="const", bufs=1) as cpool, \
             tc.tile_pool(name="refrhs", bufs=3) as refpool, \
             tc.tile_pool(name="mocorhs", bufs=2) as mocopool, \
             tc.tile_pool(name="work", bufs=2) as wpool, \
             tc.tile_pool(name="outstage", bufs=2) as opool, \
             tc.tile_pool(name="dramscratch", bufs=1, space="DRAM") as dpool, \
             tc.tile_pool(name="psum_sr", bufs=4, space="PSUM") as srpsum, \
             tc.tile_pool(name="psum_sq", bufs=2, space="PSUM") as sqpsum, \
             tc.tile_pool(name="psum_misc", bufs=2, space="PSUM") as mpsum:

            # ---- small persistent tensors -------------------------------
            iq_s = cpool.tile([128, QS], I16, tag="iq")
            nc.gpsimd.dma_start(out=iq_s[:], in_=iq_d[:].partition_broadcast(128))

            idx_s = []          # per m-tile [128,1] per-partition scalars
            for m in range(2):
                t = cpool.tile([128, 1], F32, tag=f"idx{m}", name=f"idx{m}")
                nc.scalar.dma_start(out=t[:], in_=idx_d[m * 128:(m + 1) * 128, :])
                idx_s.append(t)

            idxrow_s = cpool.tile([128, B], F32, tag="idxrow")
            nc.scalar.dma_start(out=idxrow_s[:],
                                in_=idxrow_d[:].partition_broadcast(128))

            lhsT = cpool.tile([128, KT * B], ref_dt, tag="lhsT")
            third = KT * B // 3
            for e_i, eng in enumerate((nc.sync, nc.gpsimd, nc.scalar)):
                eng.dma_start(out=lhsT[:, e_i * third:(e_i + 1) * third],
                              in_=refT_d[:, e_i * third:(e_i + 1) * third])

            ident = cpool.tile([128, 128], F32, tag="ident")
            make_identity(nc, ident[:])

            # ---- normalize q,k and build transposed copies --------------
            qnT = cpool.tile([128, B], F32, tag="qnT")
            knT = cpool.tile([128, B], F32, tag="knT")
            for (src_d, dstT) in ((q_d, qnT), (k_d, knT)):
                for m in range(2):
                    raw = wpool.tile([128, D], F32, tag="rawqk")
                    nc.scalar.dma_start(out=raw[:],
                                        in_=src_d[m * 128:(m + 1) * 128, :])
                    sqv = wpool.tile([128, D], F32, tag="sqv")
                    ssum = wpool.tile([128, 1], F32, tag="ssum")
                    nc.scalar.activation(
                        out=sqv[:], in_=raw[:],
                        func=mybir.ActivationFunctionType.Square,
                        accum_out=ssum[:])
                    rec = wpool.tile([128, 1], F32, tag="rec")
                    nc.vector.reciprocal(out=rec[:], in_=ssum[:])
                    inv = wpool.tile([128, 1], F32, tag="inv")
                    nc.scalar.sqrt(out=inv[:], in_=rec[:])
                    nrm = wpool.tile([128, D], F32, tag="nrm")
                    nc.vector.tensor_scalar_mul(nrm[:], raw[:], inv[:])
                    pt = mpsum.tile([128, 128], F32, tag='miscp', name='pt')
                    nc.tensor.transpose(pt[:], nrm[:], ident[:])
                    nc.scalar.copy(out=dstT[:, m * 128:(m + 1) * 128], in_=pt[:])

            # float32r view of qnT via a DRAM bounce (the walrus verifier
            # wants an f32r-typed producer for matmul operands)
            qnT_dram = dpool.tile([128, B], F32, tag="qnTd")
            nc.scalar.dma_start(out=qnT_dram[:], in_=qnT[:])
            qnT_r = cpool.tile([128, B], F32R, tag="qnTr")
            nc.scalar.dma_start(out=qnT_r[:], in_=qnT_dram[:].bitcast(F32R))

            # ---- score_batch + mask_batch -------------------------------
            for m in range(2):
                psb = mpsum.tile([128, B], F32, tag='miscp', name='psb')
                nc.tensor.matmul(psb[:], qnT[:, m * 128:(m + 1) * 128], knT[:],
                                 start=True, stop=True)
                sb_s = opool.tile([128, B], F32, tag="sb")
                nc.scalar.copy(out=sb_s[:], in_=psb[:])
                nc.scalar.dma_start(out=sb_d[m * 128:(m + 1) * 128, :], in_=sb_s[:])

                mb_s = opool.tile([128, B], I32, tag="mb")
                nc.vector.tensor_scalar(mb_s[:], idxrow_s[:], idx_s[m][:], None,
                                        op0=mybir.AluOpType.is_equal)
                nc.scalar.dma_start(out=maskb_d[m * 128:(m + 1) * 128, :],
                                    in_=mb_s[:])

            # ---- persistent accumulators --------------------------------
            cv_s = [cpool.tile([128, NCHD * NH * 8], F32, tag=f"cv{m}",
                               name=f"cv{m}") for m in range(2)]
            ci_s = [cpool.tile([128, NCHD * NH * 8], U32, tag=f"ci{m}",
                               name=f"ci{m}") for m in range(2)]
            mq_full = [cpool.tile([128, QS], I8, tag=f"mqf{m}",
                                  name=f"mqf{m}") for m in range(2)]

            # ---- main streaming loop ------------------------------------
            for n in range(NCHD):
                csl = slice(n * CHD, (n + 1) * CHD)

                rhs_ref = refpool.tile([128, KT * CHD], ref_dt, tag="rhsref")
                engs = (nc.sync, nc.gpsimd, nc.scalar)
                for kt in range(KT):
                    engs[kt % 3].dma_start(
                        out=rhs_ref[:, kt * CHD:(kt + 1) * CHD],
                        in_=refq_d[kt * 128:(kt + 1) * 128, csl])
                rhs_moco = mocopool.tile([128, CHD], F32R, tag="rhsmoco")
                nc.scalar.dma_start(out=rhs_moco[:],
                                    in_=moco_d[:, csl].bitcast(F32R))

                for m in range(2):
                    msl = slice(m * 128, (m + 1) * 128)
                    nc.vector.tensor_scalar(
                        mq_full[m][:, csl], iq_s[:, csl], idx_s[m][:],
                        None, op0=mybir.AluOpType.is_equal)

                    for h in range(NH):
                        hsl_t = slice(h * 512, (h + 1) * 512)         # in tile
                        hsl_g = slice(n * CHD + h * 512,
                                      n * CHD + (h + 1) * 512)        # global

                        psq = sqpsum.tile([128, 512], F32)
                        nc.tensor.matmul(psq[:], qnT_r[:, msl],
                                         rhs_moco[:, hsl_t],
                                         start=True, stop=True)

                        psr = srpsum.tile([128, 512], F32)
                        for kt in range(KT):
                            nc.tensor.matmul(
                                psr[:],
                                lhsT[:, kt * B + m * 128: kt * B + (m + 1) * 128],
                                rhs_ref[:, kt * CHD + h * 512:
                                        kt * CHD + (h + 1) * 512],
                                start=(kt == 0), stop=(kt == KT - 1))

                        sq_neg = wpool.tile([128, 512], F32, tag="sqneg")
                        nc.scalar.activation(
                            out=sq_neg[:], in_=psq[:],
                            func=mybir.ActivationFunctionType.Copy,
                            scale=-1.0)

                        prod_s = opool.tile([128, 512], F32, tag="prod")
                        nc.vector.tensor_tensor(prod_s[:], psr[:], sq_neg[:],
                                                op=mybir.AluOpType.mult)
                        nc.sync.dma_start(out=prod_d[msl, hsl_g],
                                            in_=prod_s[:])

                        sl8 = slice((n * NH + h) * 8, (n * NH + h + 1) * 8)
                        nc.vector.max(out=cv_s[m][:, sl8], in_=psr[:])
                        nc.vector.max_index(out=ci_s[m][:, sl8],
                                            in_max=cv_s[m][:, sl8],
                                            in_values=psr[:])

            for m in range(2):
                msl = slice(m * 128, (m + 1) * 128)
                nc.sync.dma_start(out=maskq_d[msl, :], in_=mq_full[m][:])
                nc.gpsimd.dma_start(out=cvals_d[msl, :], in_=cv_s[m][:])
                nc.gpsimd.dma_start(out=cidx_d[msl, :], in_=ci_s[m][:])

    nc.finalize()
    return nc, CHD, NCHD


def _get_built(mode):
    if mode not in _CACHED:
        _CACHED[mode] = _build(mode)
    return _CACHED[mode]


def kernel(q, k, ref_feats, moco_queue, ref_queue, indices, index_queue):
    global LAST_EXEC_NS
    mode = REF_MODE
    q = np.ascontiguousarray(q, dtype=np.float32)
    k = np.ascontiguousarray(k, dtype=np.float32)
    ref_feats = np.ascontiguousarray(ref_feats, dtype=np.float32)
    moco_queue = np.ascontiguousarray(moco_queue, dtype=np.float32)
    ref_queue = np.ascontiguousarray(ref_queue, dtype=np.float32)
    idx_i = np.asarray(indices)
    iq_i = np.asarray(index_queue)

    nc, CHD, NCHD = _get_built(mode)

    ref_np_dt = ml_dtypes.bfloat16 if mode == "bf16" else np.float32
    idx_f = idx_i.astype(np.float32).reshape(B, 1)
    idxrow_f = idx_i.astype(np.float32).reshape(1, B)
    refT = np.ascontiguousarray(
        ref_feats.T.astype(ref_np_dt).reshape(KT, 128, B)
        .transpose(1, 0, 2).reshape(128, KT * B))
    refq_cast = ref_queue.astype(ref_np_dt)

    in_maps = []
    for c in range(NCORES):
        sl = slice(c * QS, (c + 1) * QS)
        in_maps.append({
            "refq": np.ascontiguousarray(refq_cast[:, sl]),
            "moco": np.ascontiguousarray(moco_queue[:, sl]),
            "iq": iq_i[sl].astype(np.int16).reshape(1, QS),
            "idx": idx_f,
            "idxrow": idxrow_f,
            "q": q,
            "k": k,
            "refT": refT,
        })

    kwargs = {}
    if TRACE:
        kwargs.update(trace=True, trace_cores=list(range(NCORES)))
    res = run_bass_kernel_spmd(nc, in_maps, core_ids=list(range(NCORES)),
                               **kwargs)
    LAST_EXEC_NS = res.exec_time_ns
    outs = res.results

    score = np.empty((B, B + Q), dtype=np.float32)
    mask = np.empty((B, B + Q), dtype=np.int32)
    score[:, :B] = outs[0]["sb"]
    mask[:, :B] = outs[0]["maskb"]
    for c in range(NCORES):
        sl = slice(B + c * QS, B + (c + 1) * QS)
        score[:, sl] = outs[c]["prod"]
        mask[:, sl] = outs[c]["maskq"].astype(np.int32)

    # ---- distributed top-k merge --------------------------------------
    # candidates: per core, per CHD-chunk, top-8 (value, in-chunk index)
    vals = np.concatenate([outs[c]["cvals"] for c in range(NCORES)], axis=1)
    ncand = vals.shape[1] // NCORES
    gidx = np.concatenate(
        [(c * QS
          + (np.arange(ncand) // 8 * 512)[None, :]
          + outs[c]["cidx"].astype(np.int64))
         for c in range(NCORES)], axis=1)

    NSEL = 32
    sel = np.argsort(-vals, axis=1)[:, :NSEL]
    rows = np.arange(B)[:, None]
    sel_gidx = gidx[rows, sel]                                  # [B, NSEL]

    # exact float64 rescore of the surviving candidates
    cols = ref_queue.T[sel_gidx.reshape(-1)].reshape(B, NSEL, R)
    s64 = np.einsum("bnr,br->bn", cols.astype(np.float64),
                    ref_feats.astype(np.float64))
    # re-apply the same-id mask and kill duplicate candidates
    bad = idx_i[:, None] == iq_i[sel_gidx]
    s64[bad] = -np.inf
    order = np.argsort(-s64, axis=1, kind="stable")
    win = np.empty((B, TOPK), dtype=np.int64)
    for r in range(B):
        seen = set()
        w = []
        for j in order[r]:
            g = int(sel_gidx[r, j])
            if g not in seen and np.isfinite(s64[r, j]):
                seen.add(g)
                w.append(g)
                if len(w) == TOPK:
                    break
        win[r] = w

    score[rows, B + win] *= -1.0
    mask[rows, B + win] = 1
    return score, mask
